# revision 1
# baseline (speedup 1.0000x reference)
"""Trainium kernel for nn_PhpNetGraphTokensCombine.

Strategy (see module notes at bottom):
  - Graph message passing is converted to dense matmuls with host-built
    adjacency matrices:  a = sum_e A_e @ (h @ W_e.T) (+ degree-weighted bias).
  - Token branch: embedding gather on host; BiGRU recurrence + MLP head.
  - A Bass/Tile SPMD kernel over 8 NeuronCores executes the heavy phases
    when available; a bit-exact numpy path is the fallback.
"""
import os
import numpy as np

# Problem constants (hardcoded per task spec)
N = 2000
E = 16000
B = 16
L = 256
H = 2000
F_IN = 100
NE = 2
GH = 200
V = 50141
STEPS = 3

_BASS_CACHE = {}


def _sigmoid(x):
    out = np.empty_like(x)
    np.negative(x, out=out)
    np.exp(out, out=out)
    out += 1.0
    np.reciprocal(out, out=out)
    return out


def _gru_cell(x, h, Wih, Whh, bih, bhh):
    gi = x @ Wih.T + bih
    gh = h @ Whh.T + bhh
    ir, iz, inn = np.split(gi, 3, axis=-1)
    hr, hz, hn = np.split(gh, 3, axis=-1)
    r = _sigmoid(ir + hr)
    z = _sigmoid(iz + hz)
    n = np.tanh(inn + r * hn)
    return (1 - z) * n + z * h


def _numpy_forward(feats, tokens, src, dst, etype, batch, embed_w,
                   ggnn_W, ggnn_b, ggnn_Wih, ggnn_Whh, ggnn_bih, ggnn_bhh,
                   gru_Wih, gru_Whh, gru_bih, gru_bhh,
                   lin1_W, lin1_b, lin11_W, lin11_b, lin2_W, lin2_b):
    f32 = np.float32
    feats = feats.astype(f32)
    # Dense adjacency per edge type: A_e[d, s] = #edges s->d of type e
    A = np.zeros((NE, N, N), dtype=f32)
    deg = np.zeros((NE, N), dtype=f32)
    for e in range(NE):
        m = (etype == e)
        np.add.at(A[e], (dst[m], src[m]), 1.0)
        np.add.at(deg[e], dst[m], 1.0)

    h = np.zeros((N, H), dtype=f32)
    h[:, :F_IN] = feats
    for _ in range(STEPS):
        a = np.zeros((N, H), dtype=f32)
        for e in range(NE):
            t = h @ ggnn_W[e].T
            a += A[e] @ t + deg[e][:, None] * ggnn_b[e][None, :]
        h = _gru_cell(a, h, ggnn_Wih, ggnn_Whh, ggnn_bih, ggnn_bhh)

    # global max pool per graph (batch sorted)
    xg = np.full((B, H), -np.inf, dtype=f32)
    for g in range(B):
        m = (batch == g)
        if m.any():
            xg[g] = h[m].max(axis=0)
    xg[~np.isfinite(xg).all(axis=1)] = 0.0

    # token branch
    emb = embed_w[tokens]                    # [B, L, F_IN]
    xs = np.transpose(emb, (1, 0, 2)).astype(f32)   # [L, B, F_IN]
    xs = np.concatenate([xs, np.zeros((L, B, 2 * GH - F_IN), f32)], axis=2)
    hiddens = []
    for l in range(3):
        h0 = np.zeros((B, GH), f32)
        ys = {}
        for d in range(2):
            Wih, Whh = gru_Wih[l, d], gru_Whh[l, d]
            bih, bhh = gru_bih[l, d], gru_bhh[l, d]
            # gi for all timesteps in one matmul
            gi_all = xs.reshape(L * B, -1) @ Wih.T + bih
            gi_all = gi_all.reshape(L, B, 3 * GH)
            WhhT = np.ascontiguousarray(Whh.T)
            hh = h0.copy()
            seq = range(L) if d == 0 else range(L - 1, -1, -1)
            y = np.zeros((L, B, GH), f32)
            for t in seq:
                gh = hh @ WhhT + bhh
                gi = gi_all[t]
                r = _sigmoid(gi[:, :GH] + gh[:, :GH])
                z = _sigmoid(gi[:, GH:2 * GH] + gh[:, GH:2 * GH])
                n = np.tanh(gi[:, 2 * GH:] + r * gh[:, 2 * GH:])
                hh = (1 - z) * n + z * hh
                y[t] = hh
            ys[d] = y
            hiddens.append(hh)
        xs = np.concatenate([ys[0], ys[1]], axis=2)
    x1 = np.concatenate(hiddens, axis=1)     # [B, 6*GH]

    x = np.concatenate([xg, x1], axis=1)
    x = np.maximum(x @ lin1_W.T + lin1_b, 0)
    x = np.maximum(x @ lin11_W.T + lin11_b, 0)
    x = np.maximum(x @ lin2_W.T + lin2_b, 0)
    return x.astype(np.float32)


def kernel(**inputs):
    ins = {k: np.asarray(v) for k, v in inputs.items()}
    if os.environ.get("KERNEL_FORCE_NUMPY", "0") != "1":
        try:
            return _bass_forward(ins)
        except Exception:
            import traceback
            traceback.print_exc()
    return _numpy_forward(**ins)


# ---------------------------------------------------------------------------
# Bass/Trainium path: the GGNN (99% of FLOPs) runs on 8 NeuronCores as dense
# matmuls, column-sharded over the hidden dim with per-step AllGathers of
# transposed shards. Token BiGRU + head finish on host (latency-bound, tiny).
# ---------------------------------------------------------------------------
NP_, HP, CS, GS = 2048, 2048, 256, 768   # padded nodes/hidden, per-core shards
NC = 8


def _build_ggnn_program():
    import concourse.bacc as bacc
    import concourse.mybir as mybir
    from concourse.tile import TileContext
    from concourse.masks import make_identity
    import contextlib

    F32, BF16 = mybir.dt.float32, mybir.dt.bfloat16
    AF, ALU = mybir.ActivationFunctionType, mybir.AluOpType

    nc = bacc.Bacc("TRN2", target_bir_lowering=False, debug=False, num_devices=NC)
    h0T_in = nc.declare_dram_parameter("h0T", [CS, NP_], BF16, isOutput=False)
    h0sh_in = nc.declare_dram_parameter("h0sh", [NP_, CS], F32, isOutput=False)
    WeT_in = nc.declare_dram_parameter("WeT", [NE, HP, CS], BF16, isOutput=False)
    ATt_in = nc.declare_dram_parameter("ATt", [2 * NE * 128, NP_], BF16, isOutput=False)
    WihT_in = nc.declare_dram_parameter("WihT", [HP, GS], BF16, isOutput=False)
    WhhT_in = nc.declare_dram_parameter("WhhT", [HP, GS], BF16, isOutput=False)
    hsh_out = nc.declare_dram_parameter("hsh", [NP_, CS], F32, isOutput=True)
    KT = 16  # k/m tiles of 128

    with TileContext(nc) as tc, contextlib.ExitStack() as ctx:
        const = ctx.enter_context(tc.tile_pool(name="const", bufs=1))
        big = ctx.enter_context(tc.tile_pool(name="big", bufs=1))
        stp = ctx.enter_context(tc.tile_pool(name="stp", bufs=1))
        tpool = ctx.enter_context(tc.tile_pool(name="tpool", bufs=1))
        ghp = ctx.enter_context(tc.tile_pool(name="ghp", bufs=1))
        work = ctx.enter_context(tc.tile_pool(name="work", bufs=2))
        psS = ctx.enter_context(tc.tile_pool(name="psS", bufs=2, space="PSUM"))
        psB = ctx.enter_context(tc.tile_pool(name="psB", bufs=2, space="PSUM"))
        psT = ctx.enter_context(tc.tile_pool(name="psT", bufs=2, space="PSUM"))
        dram = ctx.enter_context(tc.tile_pool(name="dram", bufs=1, space="DRAM"))

        If32 = const.tile([128, 128], F32, tag="if32")
        make_identity(nc, If32[:])
        Ib16 = const.tile([128, 128], BF16, tag="ib16")
        nc.vector.tensor_copy(out=Ib16[:], in_=If32[:])

        WeT = [[const.tile([128, CS], BF16, tag=f"we{e}_{k}", name=f"we{e}_{k}")
                for k in range(KT)] for e in range(NE)]
        hsh = [const.tile([128, CS], F32, tag=f"hs{m}", name=f"hs{m}") for m in range(KT)]
        for k in range(KT):
            for e in range(NE):
                nc.sync.dma_start(out=WeT[e][k][:], in_=WeT_in[e, 128*k:128*(k+1), :])
            nc.sync.dma_start(out=hsh[k][:], in_=h0sh_in[128*k:128*(k+1), :])

        rg = [list(range(NC))]
        # boot: gather replicated h0T and adjacency from per-core shards
        h0T_sh = dram.tile([CS, NP_], BF16, tag="h0Tsh", name="h0Tsh")
        nc.sync.dma_start(out=h0T_sh[:], in_=h0T_in[:, :])
        h0T_full = dram.tile([HP, NP_], BF16, tag="h0Tf", name="h0Tf")
        nc.gpsimd.collective_compute("AllGather", mybir.AluOpType.bypass,
                                     replica_groups=rg, ins=[h0T_sh.opt()],
                                     outs=[h0T_full.opt()])
        ATt_sh = dram.tile([2 * NE * 128, NP_], BF16, tag="ATsh", name="ATsh")
        nc.sync.dma_start(out=ATt_sh[:], in_=ATt_in[:, :])
        ATt_full = dram.tile([16 * NE * 128, NP_], BF16, tag="ATf", name="ATf")
        nc.gpsimd.collective_compute("AllGather", mybir.AluOpType.bypass,
                                     replica_groups=rg, ins=[ATt_sh.opt()],
                                     outs=[ATt_full.opt()])
        aT_outs, hT_outs = [], []
        for s in range(STEPS):
            aT_outs.append(dram.tile([HP, NP_], BF16, tag=f"aTo{s}", name=f"aTo{s}"))
            if s < STEPS - 1:
                hT_outs.append(dram.tile([HP, NP_], BF16, tag=f"hTo{s}", name=f"hTo{s}"))

        for s in range(STEPS):
            # per-step streamed weights (share slots: Whh then Wih)
            Whh = [stp.tile([128, GS], BF16, tag=f"w{k}", name=f"whh{s}_{k}") for k in range(KT)]
            for k in range(KT):
                nc.sync.dma_start(out=Whh[k][:], in_=WhhT_in[128*k:128*(k+1), :])
            # t = h @ We.T and gh = h @ Whh.T, HT streamed in column halves
            tsb = [[tpool.tile([128, CS], BF16, tag=f"t{e}_{m}", name=f"t{s}_{e}_{m}")
                    for m in range(KT)] for e in range(NE)]
            ghsb = [ghp.tile([128, GS], BF16, tag=f"gh{m}", name=f"gh{s}_{m}") for m in range(KT)]
            for half in range(2):
                HT = [big.tile([128, 1024], BF16, tag=f"big{k}", name=f"HT{s}_{half}_{k}")
                      for k in range(KT)]
                for k in range(KT):
                    src = (h0T_full if s == 0 else hT_outs[s-1])
                    nc.sync.dma_start(out=HT[k][:],
                                      in_=src[128*k:128*(k+1), 1024*half:1024*(half+1)])
                for mm_ in range(8):
                    m = 8 * half + mm_
                    mc = slice(128*mm_, 128*(mm_+1))
                    for e in range(NE):
                        ps = psS.tile([128, CS], F32, tag="psS")
                        for k in range(KT):
                            nc.tensor.matmul(out=ps[:], lhsT=HT[k][:, mc],
                                             rhs=WeT[e][k][:], start=(k == 0), stop=(k == KT-1))
                        nc.scalar.activation(tsb[e][m][:], ps[:], AF.Copy)
                    psg = psB.tile([128, GS], F32, tag="psB")
                    for k in range(KT):
                        nc.tensor.matmul(out=psg[:, 0:512], lhsT=HT[k][:, mc],
                                         rhs=Whh[k][:, 0:512], start=(k == 0), stop=(k == KT-1))
                        nc.tensor.matmul(out=psg[:, 512:GS], lhsT=HT[k][:, mc],
                                         rhs=Whh[k][:, 512:GS], start=(k == 0), stop=(k == KT-1))
                    nc.scalar.activation(ghsb[m][:], psg[:], AF.Copy)
            # 4. a = sum_e A_e @ t_e ; 5. transpose shard
            aTsh = [work.tile([128, NP_], BF16, tag=f"aTs{h}", name=f"aTs{s}_{h}") for h in range(2)]
            for m in range(KT):
                ps = psS.tile([128, CS], F32, tag="psS")
                for e in range(NE):
                    slab = work.tile([128, NP_], BF16, tag="aslab", name=f"aslab{s}_{e}_{m}")
                    nc.sync.dma_start(out=slab[:], in_=ATt_full[(NE*m+e)*128:(NE*m+e+1)*128, :])
                    for k in range(KT):
                        nc.tensor.matmul(out=ps[:], lhsT=slab[:, 128*k:128*(k+1)],
                                         rhs=tsb[e][k][:], start=(e == 0 and k == 0),
                                         stop=(e == NE-1 and k == KT-1))
                ash = work.tile([128, CS], BF16, tag="ash", name=f"ash{s}_{m}")
                nc.scalar.activation(ash[:], ps[:], AF.Copy)
                for h in range(2):
                    pst = psT.tile([128, 128], BF16, tag="psT", name=f"psta{s}_{m}_{h}")
                    nc.tensor.transpose(out=pst[:], in_=ash[:, 128*h:128*(h+1)], identity=Ib16[:])
                    nc.scalar.activation(aTsh[h][:, 128*m:128*(m+1)], pst[:], AF.Copy)
            # 6. AllGather aT
            aT_in = dram.tile([CS, NP_], BF16, tag="aTin", name=f"aTin{s}")
            for h in range(2):
                nc.sync.dma_start(out=aT_in[128*h:128*(h+1), :], in_=aTsh[h][:])
            nc.gpsimd.collective_compute("AllGather", mybir.AluOpType.bypass,
                                         replica_groups=rg, ins=[aT_in.opt()],
                                         outs=[aT_outs[s].opt()])
            # 7.+8. gi (aT slabs in column halves, reusing big slots) + gates
            Wih = [stp.tile([128, GS], BF16, tag=f"w{k}", name=f"wi{s}_{k}") for k in range(KT)]
            for k in range(KT):
                nc.sync.dma_start(out=Wih[k][:], in_=WihT_in[128*k:128*(k+1), :])
            hTsh = [work.tile([128, NP_], BF16, tag=f"hTs{h}", name=f"hTs{s}_{h}") for h in range(2)]
            for half in range(2):
              ATk = [big.tile([128, 1024], BF16, tag=f"big{k}", name=f"ATk{s}_{half}_{k}")
                     for k in range(KT)]
              for k in range(KT):
                nc.sync.dma_start(out=ATk[k][:],
                                  in_=aT_outs[s][128*k:128*(k+1), 1024*half:1024*(half+1)])
              for mm_ in range(8):
                m = 8 * half + mm_
                mc = slice(128*mm_, 128*(mm_+1))
                ps = psB.tile([128, GS], F32, tag="psB")
                for k in range(KT):
                    nc.tensor.matmul(out=ps[:, 0:512], lhsT=ATk[k][:, mc],
                                     rhs=Wih[k][:, 0:512], start=(k == 0), stop=(k == KT-1))
                    nc.tensor.matmul(out=ps[:, 512:GS], lhsT=ATk[k][:, mc],
                                     rhs=Wih[k][:, 512:GS], start=(k == 0), stop=(k == KT-1))
                Grz = work.tile([128, 512], F32, tag="grz", name=f"grz{s}_{m}")
                nc.vector.tensor_tensor(out=Grz[:], in0=ps[:, 0:512], in1=ghsb[m][:, 0:512], op=ALU.add)
                RZ = work.tile([128, 512], F32, tag="rz", name=f"rz{s}_{m}")
                nc.scalar.activation(RZ[:], Grz[:], AF.Sigmoid)
                u = work.tile([128, CS], F32, tag="u", name=f"u{s}_{m}")
                nc.vector.tensor_tensor(out=u[:], in0=RZ[:, 0:CS], in1=ghsb[m][:, 512:GS], op=ALU.mult)
                npre = work.tile([128, CS], F32, tag="npre", name=f"npre{s}_{m}")
                nc.vector.tensor_tensor(out=npre[:], in0=u[:], in1=ps[:, 512:GS], op=ALU.add)
                nn = work.tile([128, CS], F32, tag="nn", name=f"nn{s}_{m}")
                nc.scalar.activation(nn[:], npre[:], AF.Tanh)
                dd = work.tile([128, CS], F32, tag="dd", name=f"dd{s}_{m}")
                nc.vector.tensor_tensor(out=dd[:], in0=hsh[m][:], in1=nn[:], op=ALU.subtract)
                ee = work.tile([128, CS], F32, tag="ee", name=f"ee{s}_{m}")
                nc.vector.tensor_tensor(out=ee[:], in0=RZ[:, CS:512], in1=dd[:], op=ALU.mult)
                nc.vector.tensor_tensor(out=hsh[m][:], in0=nn[:], in1=ee[:], op=ALU.add)
                if s < STEPS - 1:
                    for h in range(2):
                        pst = psT.tile([128, 128], F32, tag="psT", name=f"psth{s}_{m}_{h}")
                        nc.tensor.transpose(out=pst[:], in_=hsh[m][:, 128*h:128*(h+1)], identity=If32[:])
                        nc.scalar.activation(hTsh[h][:, 128*m:128*(m+1)], pst[:], AF.Copy)
            # 9. AllGather h
            if s < STEPS - 1:
                hT_in = dram.tile([CS, NP_], BF16, tag="hTin", name=f"hTin{s}")
                for h in range(2):
                    nc.sync.dma_start(out=hT_in[128*h:128*(h+1), :], in_=hTsh[h][:])
                nc.gpsimd.collective_compute("AllGather", mybir.AluOpType.bypass,
                                             replica_groups=rg, ins=[hT_in.opt()],
                                             outs=[hT_outs[s].opt()])
        for m in range(KT):
            nc.sync.dma_start(out=hsh_out[128*m:128*(m+1), :], in_=hsh[m][:])
    nc.compile()
    return nc


def _run_spmd(nc, in_maps):
    try:
        return _run_spmd_cached(nc, in_maps)
    except Exception:
        from concourse.bass_utils import run_bass_kernel_spmd
        return run_bass_kernel_spmd(nc, in_maps, list(range(NC)), trace=False).results


def _run_spmd_cached(nc, in_maps):
    """Compile-once PJRT runner: avoids re-tracing jax.jit on repeat calls."""
    import jax
    import concourse.mybir as mybir
    from jax.sharding import Mesh, PartitionSpec
    from jax.experimental.shard_map import shard_map
    from concourse.bass2jax import _bass_exec_p, install_neuronx_cc_hook, \
        partition_id_tensor

    if "runner" not in _BASS_CACHE:
        install_neuronx_cc_hook()
        pname = nc.partition_id_tensor.name if nc.partition_id_tensor else None
        in_names, out_names, out_avals, zero_outs = [], [], [], []
        for alloc in nc.m.functions[0].allocations:
            if not isinstance(alloc, mybir.MemoryLocationSet):
                continue
            name = alloc.memorylocations[0].name
            if alloc.kind == "ExternalInput":
                if name != pname:
                    in_names.append(name)
            elif alloc.kind == "ExternalOutput":
                out_names.append(name)
                shape, dt = tuple(alloc.tensor_shape), mybir.dt.np(alloc.dtype)
                out_avals.append(jax.core.ShapedArray(shape, dt))
                zero_outs.append(np.zeros(shape, dt))
        all_in = list(in_names) + list(out_names)
        if pname is not None:
            all_in.append(pname)

        def _body(*args):
            ops = list(args)
            if pname is not None:
                ops.append(partition_id_tensor())
            return tuple(_bass_exec_p.bind(
                *ops, out_avals=tuple(out_avals), in_names=tuple(all_in),
                out_names=tuple(out_names), lowering_input_output_aliases=(),
                sim_require_finite=True, sim_require_nnan=True, nc=nc))

        mesh = Mesh(np.asarray(jax.devices()[:NC]), ("core",))
        nio = len(in_names) + len(out_names)
        fn = jax.jit(shard_map(_body, mesh=mesh,
                               in_specs=(PartitionSpec("core"),) * nio,
                               out_specs=(PartitionSpec("core"),) * len(out_names),
                               check_rep=False), keep_unused=True)
        _BASS_CACHE["runner"] = (fn, in_names, out_names, zero_outs)

    fn, in_names, out_names, zero_outs = _BASS_CACHE["runner"]
    concat_in = [np.concatenate([np.asarray(m[nm]) for m in in_maps], axis=0)
                 for nm in in_names]
    concat_zero = [np.concatenate([z] * NC, axis=0) for z in zero_outs]
    outs = fn(*concat_in, *concat_zero)
    res = [dict() for _ in range(NC)]
    for i, nm in enumerate(out_names):
        arr = np.asarray(outs[i])
        step = arr.shape[0] // NC
        for c in range(NC):
            res[c][nm] = arr[c*step:(c+1)*step]
    return res


def _bass_forward(ins):
    import ml_dtypes
    bf16 = ml_dtypes.bfloat16
    f32 = np.float32
    for bname in ("ggnn_b", "ggnn_bih", "ggnn_bhh"):
        if np.any(ins[bname]):
            raise ValueError("nonzero ggnn bias: fallback")

    src, dst, etype, batch = ins["src"], ins["dst"], ins["etype"], ins["batch"]
    # host prep: padded transposed tensors
    h0 = np.zeros((NP_, HP), f32)
    h0[:N, :F_IN] = ins["feats"]
    h0T = np.ascontiguousarray(h0.T).astype(bf16)

    A = np.zeros((NE, NP_, NP_), f32)
    for e in range(NE):
        m = (etype == e)
        np.add.at(A[e], (dst[m], src[m]), 1.0)
    # ATt_m[m, e, p, k*128+j] = A_e.T[128k+p, 128m+j]; per-core shard = 2 m-tiles
    ATt_m = np.ascontiguousarray(
        A.transpose(0, 2, 1).reshape(NE, 16, 128, 16, 128).transpose(3, 0, 2, 1, 4)
        .reshape(16, NE * 128, NP_)).astype(bf16)

    Wp = np.zeros((NE, HP, HP), f32)
    Wp[:, :H, :H] = ins["ggnn_W"]
    Wihp = np.zeros((3 * HP, HP), f32)
    Whhp = np.zeros((3 * HP, HP), f32)
    for j in range(3):
        Wihp[j*HP:j*HP+H, :H] = ins["ggnn_Wih"][j*H:(j+1)*H]
        Whhp[j*HP:j*HP+H, :H] = ins["ggnn_Whh"][j*H:(j+1)*H]

    in_maps = []
    for c in range(NC):
        cols = slice(CS*c, CS*(c+1))
        grows = np.r_[CS*c:CS*(c+1), HP+CS*c:HP+CS*(c+1), 2*HP+CS*c:2*HP+CS*(c+1)]
        in_maps.append({
            "h0T": np.ascontiguousarray(h0T[CS*c:CS*(c+1), :]),
            "h0sh": np.ascontiguousarray(h0[:, cols]),
            "WeT": np.ascontiguousarray(Wp[:, cols, :].transpose(0, 2, 1)).astype(bf16),
            "ATt": ATt_m[2*c:2*(c+1)].reshape(2 * NE * 128, NP_),
            "WihT": np.ascontiguousarray(Wihp[grows, :].T).astype(bf16),
            "WhhT": np.ascontiguousarray(Whhp[grows, :].T).astype(bf16),
        })

    key = "ggnn"
    if key not in _BASS_CACHE:
        _BASS_CACHE[key] = _build_ggnn_program()

    # run the device GGNN concurrently with the host token branch (they are
    # independent until the head)
    import threading
    dev = {}

    def _dev_work():
        try:
            dev["res"] = _run_spmd(_BASS_CACHE[key], in_maps)
        except Exception as exc:  # surfaced after join
            dev["err"] = exc

    th = threading.Thread(target=_dev_work)
    th.start()

    emb = ins["embed_w"][ins["tokens"]]
    xs = np.transpose(emb, (1, 0, 2)).astype(f32)
    xs = np.concatenate([xs, np.zeros((L, B, 2*GH - F_IN), f32)], axis=2)
    hiddens = []
    for l in range(3):
        ys = {}
        for d in range(2):
            Wih, Whh = ins["gru_Wih"][l, d], ins["gru_Whh"][l, d]
            bih, bhh = ins["gru_bih"][l, d], ins["gru_bhh"][l, d]
            gi_all = (xs.reshape(L*B, -1) @ Wih.T + bih).reshape(L, B, 3*GH)
            WhhT = np.ascontiguousarray(Whh.T)
            hh = np.zeros((B, GH), f32)
            seq = range(L) if d == 0 else range(L-1, -1, -1)
            y = np.zeros((L, B, GH), f32)
            for t in seq:
                gh = hh @ WhhT + bhh
                gi = gi_all[t]
                r = _sigmoid(gi[:, :GH] + gh[:, :GH])
                z = _sigmoid(gi[:, GH:2*GH] + gh[:, GH:2*GH])
                n = np.tanh(gi[:, 2*GH:] + r * gh[:, 2*GH:])
                hh = (1 - z) * n + z * hh
                y[t] = hh
            ys[d] = y
            hiddens.append(hh)
        xs = np.concatenate([ys[0], ys[1]], axis=2)
    x1 = np.concatenate(hiddens, axis=1)

    th.join()
    if "err" in dev:
        raise dev["err"]
    res = dev["res"]
    h = np.zeros((NP_, HP), f32)
    for c in range(NC):
        h[:, CS*c:CS*(c+1)] = res[c]["hsh"]
    h = h[:N, :H]
    xg = np.zeros((B, H), f32)
    for g in range(B):
        m = (batch == g)
        if m.any():
            xg[g] = h[m].max(axis=0)

    x = np.concatenate([xg, x1], axis=1)
    x = np.maximum(x @ ins["lin1_W"].T + ins["lin1_b"], 0)
    x = np.maximum(x @ ins["lin11_W"].T + ins["lin11_b"], 0)
    x = np.maximum(x @ ins["lin2_W"].T + ins["lin2_b"], 0)
    return x.astype(np.float32)

